# revision 15
# baseline (speedup 1.0000x reference)
"""Bahdanau attention Trainium2 kernel (mask-sparse).

Shapes (full): hidden (32,1024) f32, encoder_outputs (32,2048,1024) f32,
mask (32,2048) i32, W_h/W_e (1024,1024) f32, b_h/b_e/v (1024,) f32.
Outputs: context (32,1024) f32, attention_weights (32,2048) f32.

Sharding: data-parallel over batch B across 8 cores (4 batches/core);
projection weights replicated.

Mask sparsity: positions with mask==0 get attention weight exactly 0, so
they contribute nothing to either output. The host compacts each batch's
unmasked encoder rows (block-sparse descriptor metadata computed from the
mask), pads to a uniform tile count NUP (multiple of 128), and the device
runs a dense pipeline over only those rows (~half the work for a
Bernoulli(1/2) mask). Pad rows are zeroed by the same multiply that used
to apply the mask. Compacted weights are scattered back on host.

Per-core pipeline (all big compute in bf16 on the PE):
  h_projT = W_h^T @ hidden^T + b_h + b_e          (tiny, startup)
  per s-block of <=1024 gathered rows:
    natbf  <- gpsimd cast-DMA f32->bf16 of gathered enc rows [128,1024] x nt
    bt     <- xbar transpose (3D out) of natbf           [128,8(et),1024(s)]
    e_projT[h,s] = sum_et W_e[et]^T @ bt[et]  (PSUM f32)
    energy = tanh(e_projT + bias)  (ACT, per-partition bias)
    logits[s] = sum_ht v[ht]^T @ energy[ht]   (M=1 matmuls, lagged 1 h-tile)
    p = exp(logits - |v|_1) * padmask  (no row max needed: |logit| <= |v|_1)
    pT <- tiny DRAM round-trip xbar ( [16,128] -> [128,16] )
    ctx += sum_j pT[:,j]^T @ natbf[j]          (M=1 matmuls, deferred 1 block)
  ctx_out = ctx * (1/sum p);  w_out = p * (1/sum p)  (w scattered on host)
"""

import sys

for _p in ("/opt/trn_rl_repo", "/root/.axon_site/_ro/trn_rl_repo"):
    if _p not in sys.path:
        sys.path.insert(0, _p)

import numpy as np

import concourse.bacc as bacc
import concourse.mybir as mybir
import concourse.tile as tile

F32 = mybir.dt.float32
BF16 = mybir.dt.bfloat16
I32 = mybir.dt.int32
AF = mybir.ActivationFunctionType

N_CORES = 8
B_FULL, S, D = 32, 2048, 1024
B_LOC = B_FULL // N_CORES          # 4 batches per core
SB = 1024                          # max s-block size (8 tiles of 128)
NET = D // 128                     # 8 e/h tiles


def make_blocks(nup):
    """Split each batch's nup gathered rows into blocks of <=8 tiles.

    Returns list of (b, kblk, t0, nt, first, last): batch, block index in
    batch, first tile, tile count, is-first/last block of the batch.
    """
    ntiles = nup // 128
    blocks = []
    for b in range(B_LOC):
        nblk = (ntiles + 7) // 8
        for kblk in range(nblk):
            t0 = kblk * 8
            nt = min(8, ntiles - t0)
            blocks.append((b, kblk, t0, nt, kblk == 0, kblk == nblk - 1))
    return blocks


def emit(tc, outs, ins, shift, nup, repeat=1):
    nc = tc.nc
    ctx_out, w_out = outs
    hidden, enc, padmask, W_h, b_h, W_e, b_e, v = ins
    blocks = make_blocks(nup)
    NG = len(blocks)
    nblk_b = NG // B_LOC

    from contextlib import ExitStack
    stack = ExitStack()
    pool = stack.enter_context(tc.tile_pool(name="sb", bufs=1))
    dpool = stack.enter_context(tc.tile_pool(name="dr", bufs=1, space="DRAM"))
    ppool = stack.enter_context(tc.tile_pool(name="ps", bufs=1, space="PSUM"))

    # ---- constant / weight setup ----
    negshift = pool.tile([128, 1], F32, name="negshift")
    nc.gpsimd.memset(negshift[:], -float(shift))

    hTbf = pool.tile([128, NET, B_LOC], BF16, name="hTbf")
    for b in range(B_LOC):
        nc.gpsimd.dma_start(
            out=hTbf[:, :, b], in_=hidden[b].rearrange("(k p) -> p k", p=128)
        )
    vbf = pool.tile([128, NET], BF16, name="vbf")
    nc.gpsimd.dma_start(out=vbf[:], in_=v.rearrange("(et p) -> p et", p=128))
    # Weights via HWDGE f32 loads + DVE cast: keeps the SWDGE rail free for
    # the enc cast-DMA preloads during warmup. Layout [p, et, h], row e=et*128+p.
    whbf = pool.tile([128, NET, D], BF16, name="whbf")
    webf = pool.tile([128, NET, D], BF16, name="webf")
    for (wsrc, wdst) in ((W_h, whbf), (W_e, webf)):
        for et in range(NET):
            wf = pool.tile([128, D], F32, name=f"wf{et}", tag="wf32", bufs=2)
            nc.sync.dma_start(out=wf[:], in_=wsrc[et * 128:(et + 1) * 128, :])
            nc.vector.tensor_copy(wdst[:, et, :], wf[:])

    bh_sb = pool.tile([128, NET], F32, name="bh_sb")
    nc.sync.dma_start(out=bh_sb[:], in_=b_h.rearrange("(k p) -> p k", p=128))
    be_sb = pool.tile([128, NET], F32, name="be_sb")
    nc.sync.dma_start(out=be_sb[:], in_=b_e.rearrange("(k p) -> p k", p=128))
    bsum = pool.tile([128, NET], F32, name="bsum")
    nc.vector.tensor_add(bsum[:], bh_sb[:], be_sb[:])

    # ---- h_proj: hpb[p, m, b] = (hidden @ W_h)[b, m*128+p] + b_h + b_e ----
    hpb = pool.tile([128, NET, B_LOC], F32, name="hpb")
    for m in range(NET):
        psh = ppool.tile([128, B_LOC], F32, name=f"psh{m}", tag="vd", bufs=1)
        for k in range(NET):
            nc.tensor.matmul(
                psh[:],
                whbf[:, k, m * 128:(m + 1) * 128],
                hTbf[:, k, :],
                start=(k == 0), stop=(k == NET - 1),
            )
        nc.scalar.activation(hpb[:, m, :], psh[:], AF.Identity, bias=bsum[:, m:m + 1])

    # ---- pipelined main loop over gathered s-blocks ----
    natbf = {}     # g -> list of nt [128,1024] bf16 tiles
    btt = {}       # g -> [128, 8, 1024] bf16 block-transposed
    pTt = {}       # g -> [128, 16] bf16
    dparts = {}    # b -> [1, nblk_b] f32
    p32dram = {}   # b -> [1, nup] f32 DRAM
    psctx = {}     # b -> [1, 1024] f32 PSUM

    def emit_preload(g):
        b, kblk, t0, nt, first, last = blocks[g]
        tiles = []
        for j in range(nt):
            t = pool.tile([128, SB], BF16, name=f"nat{g}_{j}", tag="natbf", bufs=32)
            s0 = (t0 + j) * 128
            nc.gpsimd.dma_start(out=t[:], in_=enc[b, s0:s0 + 128, :])
            tiles.append(t)
        natbf[g] = tiles

    def emit_transpose(g):
        nt = blocks[g][3]
        bt = pool.tile([128, NET, SB], BF16, name=f"bt{g}", tag="bt", bufs=2)
        for j in range(nt):
            nc.sync.dma_start_transpose(
                out=bt[:, :, j * 128:(j + 1) * 128], in_=natbf[g][j][:]
            )
        btt[g] = bt

    def emit_compute(g):
        b, kblk, t0, nt, first, last = blocks[g]
        blen = nt * 128
        bt = btt[g]
        energy = []
        psv = ppool.tile([1, SB], F32, name=f"psv{g}", tag="vd", bufs=1)

        def emit_vdot(ht):
            for o in range(0, blen, 512):
                n = min(512, blen - o)
                nc.tensor.matmul(
                    psv[0:1, o:o + n],
                    vbf[:, ht:ht + 1],
                    energy[ht][:, o:o + n],
                    start=(ht == 0), stop=(ht == NET - 1),
                    skip_group_check=True,
                )

        for ht in range(NET):
            ps = ppool.tile([128, SB], F32, name=f"pe{g}_{ht}", tag="pe", bufs=2)
            for o in range(0, blen, 512):
                n = min(512, blen - o)
                for et in range(NET):
                    nc.tensor.matmul(
                        ps[:, o:o + n],
                        webf[:, et, ht * 128:(ht + 1) * 128],
                        bt[:, et, o:o + n],
                        start=(et == 0), stop=(et == NET - 1),
                        skip_group_check=True,
                    )
            en = pool.tile([128, SB], BF16, name=f"en{g}_{ht}", tag="en", bufs=4)
            nc.scalar.activation(
                en[:, :blen], ps[:, :blen], AF.Tanh, bias=hpb[:, ht, b:b + 1]
            )
            energy.append(en)
            # v-dot lags one h-tile behind e_proj so tanh(ht) overlaps
            # e_proj(ht+1) on the PE and energy slots free continuously
            if ht >= 1:
                emit_vdot(ht - 1)
        emit_vdot(NET - 1)

        # p = exp(logits - shift) * padmask
        mi = pool.tile([1, SB], I32, name=f"mi{g}", tag="mi", bufs=2)
        nc.sync.dma_start(out=mi[0:1, :blen], in_=padmask[b, t0 * 128:t0 * 128 + blen])
        mf = pool.tile([1, SB], F32, name=f"mf{g}", tag="mf", bufs=2)
        nc.vector.tensor_copy(mf[0:1, :blen], mi[0:1, :blen])
        p32 = pool.tile([1, SB], F32, name=f"p32{g}", tag="p32", bufs=2)
        nc.scalar.activation(
            p32[0:1, :blen], psv[0:1, :blen], AF.Exp, bias=negshift[0:1, :]
        )
        nc.vector.tensor_mul(p32[0:1, :blen], p32[0:1, :blen], mf[0:1, :blen])
        if first:
            dparts[b] = pool.tile([1, nblk_b], F32, name=f"dp{b}", tag="dp", bufs=2)
            p32dram[b] = dpool.tile([1, nup], F32, name=f"p32d{b}", tag="p32d", bufs=2)
        nc.vector.reduce_sum(
            dparts[b][0:1, kblk:kblk + 1], p32[0:1, :blen], axis=mybir.AxisListType.X
        )
        pbf = pool.tile([1, SB], BF16, name=f"pbf{g}", tag="pbf", bufs=2)
        nc.vector.tensor_copy(pbf[0:1, :blen], p32[0:1, :blen])
        nc.sync.dma_start(
            out=p32dram[b][0:1, t0 * 128:t0 * 128 + blen], in_=p32[0:1, :blen]
        )

        # transpose p via tiny DRAM round trip (xbar needs >=16 rows; rows
        # nt..15 are junk and never read)
        pd = dpool.tile([16, 128], BF16, name=f"pd{g}", tag="pd", bufs=2)
        nc.sync.dma_start(out=pd[0:nt, :], in_=pbf[0:1, :blen])
        pT = pool.tile([128, 16], BF16, name=f"pT{g}", tag="pT", bufs=2)
        nc.sync.dma_start_transpose(out=pT[:], in_=pd[:])
        pTt[g] = pT

    def emit_ctx(g):
        b, kblk, t0, nt, first, last = blocks[g]
        if first:
            psctx[b] = ppool.tile([1, D], F32, name=f"psctx{b}", tag="ctx", bufs=1)
        pc = psctx[b]
        for j in range(nt):
            for half in range(2):
                o = half * 512
                nc.tensor.matmul(
                    pc[0:1, o:o + 512],
                    pTt[g][:, j:j + 1],
                    natbf[g][j][:, o:o + 512],
                    start=(first and j == 0), stop=(last and j == nt - 1),
                    skip_group_check=True,
                )

    def emit_batchend(b):
        dsum = pool.tile([1, 1], F32, name=f"ds{b}", tag="ds", bufs=2)
        nc.vector.reduce_sum(dsum[:], dparts[b][:], axis=mybir.AxisListType.X)
        rcp = pool.tile([1, 1], F32, name=f"rcp{b}", tag="rcp", bufs=2)
        nc.vector.reciprocal(rcp[:], dsum[:])
        ctxsb = pool.tile([1, D], F32, name=f"ctxsb{b}", tag="ctxsb", bufs=2)
        nc.scalar.activation(ctxsb[:], psctx[b][:], AF.Copy, scale=rcp[0:1, :])
        nc.sync.dma_start(out=ctx_out[b, :], in_=ctxsb[:])
        for o in range(0, nup, SB):
            n = min(SB, nup - o)
            wt = pool.tile([1, SB], F32, name=f"wt{b}_{o}", tag="wt", bufs=2)
            nc.sync.dma_start(out=wt[0:1, :n], in_=p32dram[b][0:1, o:o + n])
            nc.vector.tensor_scalar_mul(wt[0:1, :n], wt[0:1, :n], rcp[0:1, :])
            nc.sync.dma_start(out=w_out[b, o:o + n], in_=wt[0:1, :n])

    for _rep in range(repeat):
        natbf.clear(); btt.clear(); pTt.clear()
        dparts.clear(); p32dram.clear(); psctx.clear()
        for i in range(-2, NG + 1):
            if 0 <= i + 2 < NG:
                emit_preload(i + 2)
            if 0 <= i + 1 < NG:
                emit_transpose(i + 1)
            if 0 <= i < NG:
                emit_compute(i)
            if 0 <= i - 1 < NG:
                emit_ctx(i - 1)
                if blocks[i - 1][5]:
                    emit_batchend(blocks[i - 1][0])

    stack.close()


def build_nc(shift, nup=S, repeat=1):
    nc = bacc.Bacc("TRN2", target_bir_lowering=False, debug=False)
    ins = [
        nc.dram_tensor("hidden", [B_LOC, D], F32, kind="ExternalInput").ap(),
        nc.dram_tensor("enc_g", [B_LOC, nup, D], F32, kind="ExternalInput").ap(),
        nc.dram_tensor("padmask", [B_LOC, nup], I32, kind="ExternalInput").ap(),
        nc.dram_tensor("W_h", [D, D], F32, kind="ExternalInput").ap(),
        nc.dram_tensor("b_h", [D], F32, kind="ExternalInput").ap(),
        nc.dram_tensor("W_e", [D, D], F32, kind="ExternalInput").ap(),
        nc.dram_tensor("b_e", [D], F32, kind="ExternalInput").ap(),
        nc.dram_tensor("v", [D], F32, kind="ExternalInput").ap(),
    ]
    outs = [
        nc.dram_tensor("ctx_out", [B_LOC, D], F32, kind="ExternalOutput").ap(),
        nc.dram_tensor("w_out", [B_LOC, nup], F32, kind="ExternalOutput").ap(),
    ]
    with tile.TileContext(nc) as tc:
        emit(tc, outs, ins, shift, nup, repeat=repeat)
    nc.compile()
    return nc


def prepare_sparse(encoder_outputs, mask):
    """Host-side block-sparse compaction: gather unmasked rows per batch.

    Returns (enc_g [B,nup,D] f32, padmask [B,nup] i32, idx list, nup).
    """
    idx = [np.nonzero(mask[b] != 0)[0] for b in range(B_FULL)]
    maxcnt = max(1, max(len(ix) for ix in idx))
    nup = min(S, ((maxcnt + 127) // 128) * 128)
    enc_g = np.zeros((B_FULL, nup, D), dtype=np.float32)
    padmask = np.zeros((B_FULL, nup), dtype=np.int32)
    for b in range(B_FULL):
        c = len(idx[b])
        enc_g[b, :c] = encoder_outputs[b, idx[b]]
        padmask[b, :c] = 1
    return enc_g, padmask, idx, nup


def kernel(hidden, encoder_outputs, mask, W_h, b_h, W_e, b_e, v):
    from concourse.bass_utils import run_bass_kernel_spmd

    hidden = np.ascontiguousarray(np.asarray(hidden, dtype=np.float32))
    encoder_outputs = np.asarray(encoder_outputs, dtype=np.float32)
    mask = np.asarray(mask, dtype=np.int32)
    W_h = np.ascontiguousarray(np.asarray(W_h, dtype=np.float32))
    b_h = np.ascontiguousarray(np.asarray(b_h, dtype=np.float32))
    W_e = np.ascontiguousarray(np.asarray(W_e, dtype=np.float32))
    b_e = np.ascontiguousarray(np.asarray(b_e, dtype=np.float32))
    v = np.ascontiguousarray(np.asarray(v, dtype=np.float32))

    enc_g, padmask, idx, nup = prepare_sparse(encoder_outputs, mask)
    shift = float(np.abs(v).sum())
    nc = build_nc(shift, nup=nup)

    in_maps = []
    for c in range(N_CORES):
        sl = slice(c * B_LOC, (c + 1) * B_LOC)
        in_maps.append({
            "hidden": np.ascontiguousarray(hidden[sl]),
            "enc_g": np.ascontiguousarray(enc_g[sl]),
            "padmask": np.ascontiguousarray(padmask[sl]),
            "W_h": W_h, "b_h": b_h, "W_e": W_e, "b_e": b_e, "v": v,
        })

    res = run_bass_kernel_spmd(nc, in_maps, list(range(N_CORES)))
    context = np.concatenate([res.results[c]["ctx_out"] for c in range(N_CORES)], axis=0)
    w_g = np.concatenate([res.results[c]["w_out"] for c in range(N_CORES)], axis=0)
    weights = np.zeros((B_FULL, S), dtype=np.float32)
    for b in range(B_FULL):
        weights[b, idx[b]] = w_g[b, :len(idx[b])]
    return (context, weights)
